# revision 9
# baseline (speedup 1.0000x reference)
"""Trainium2 Bass kernel for MAR-block-missingness (segment_reduce).

Computes, for X [8192, 8192] f32 with sorted row_ids/col_ids in [0, 32):
  propensity = sigmoid(MLP(block_mean(X)))  [32, 32]
  row_cumsum, col_cumsum                    [33] int32 (index bookkeeping)

Strategy (8 NeuronCores, SPMD):
  - X row-sharded: core c gets rows [1024c, 1024(c+1)).
  - Stage 1 (PE): row-block segment sums via matmul with the one-hot
    row-block matrix S (host-built from row_ids): psum[32, 512] +=
    S_tile.T @ X_tile, accumulated over the 8 row tiles of the shard.
    float32r streaming mode for 1 cycle/row.
  - Stage 2 (DVE): column-block segment sums via reduce_sum over the
    sorted col-block ranges (compile-time baked from col_ids), added
    into a [32, 32] accumulator.
  - AllReduce the [32, 32] partial block sums across the 8 cores.
  - MLP (replicated on every core): x = blk * inv_cnt laid out as
    [128, 8] columns; h1 = relu(W1.T x + b1) as [100, 1] column;
    h2 = relu(W2.T h1 + b2); out = sigmoid(W3.T h2 + b3) as [128, 8].
"""

import os
import time

import numpy as np

N = 8192
T = 8192
RB = 32
CB = 32
HID = 100
NCORES = 8
RPC = N // NCORES      # rows per core (1024)
RT = RPC // 128        # 128-row tiles per core (8)
QW = 512               # column unit width (0.25 MB DMA tiles)
NQ = T // QW           # 4
CH = 512               # psum chunk width (one PSUM bank of f32)
NCH = QW // CH         # 4

USE_F32R = True        # float32r streaming matmul (4x faster than f32)
DEVICE_MLP = True      # all-reduce + MLP on device; else host finishes

_cache = {}
LAST_PERF = {"exec_time_ns": None, "run_seconds": None}


def _build_program(col_cum):
    """Build + compile the SPMD Bass program. col_cum: [33] col-block
    boundaries (cumsum of col bincounts), baked in as compile-time
    reduce ranges."""
    import concourse.bacc as bacc
    import concourse.mybir as mybir
    import concourse.tile as tile

    F32 = mybir.dt.float32
    F32R = mybir.dt.float32r

    nc = bacc.Bacc(
        "TRN2",
        target_bir_lowering=False,
        debug=False,
        enable_asserts=False,
        num_devices=NCORES,
    )

    x_d = nc.dram_tensor("x", [RPC, T], F32, kind="ExternalInput").ap()
    s_d = nc.dram_tensor("s", [RPC, RB], F32, kind="ExternalInput").ap()
    w1_d = nc.dram_tensor("w1", [RB * CB, HID], F32, kind="ExternalInput").ap()
    w2_d = nc.dram_tensor("w2", [HID, HID], F32, kind="ExternalInput").ap()
    w3_d = nc.dram_tensor("w3", [HID, RB * CB], F32, kind="ExternalInput").ap()
    b1_d = nc.dram_tensor("b1", [HID, 1], F32, kind="ExternalInput").ap()
    b2_d = nc.dram_tensor("b2", [HID, 1], F32, kind="ExternalInput").ap()
    b3_d = nc.dram_tensor("b3", [128, 8], F32, kind="ExternalInput").ap()
    ic_d = nc.dram_tensor("ic", [128, 8], F32, kind="ExternalInput").ap()
    blk_d = nc.dram_tensor("blkpart", [RB, CB], F32, kind="ExternalOutput").ap()
    if DEVICE_MLP:
        prop_d = nc.dram_tensor("prop", [128, 8], F32, kind="ExternalOutput").ap()

    with tile.TileContext(nc) as tc:
        with (
            tc.tile_pool(name="sb", bufs=1) as sb,
            tc.tile_pool(name="xp", bufs=16) as xp,
            tc.tile_pool(name="ps", bufs=6, space="PSUM") as ps,
            tc.tile_pool(name="mps", bufs=2, space="PSUM") as mps,
            tc.tile_pool(name="dp", bufs=1, space="DRAM") as dp,
        ):
            # --- constants: one-hot row-block tiles (one DMA, side by side) ---
            s_all = sb.tile([128, RT * RB], F32R, name="s_all", tag="s_all", bufs=1)
            nc.sync.dma_start(
                out=s_all[:].rearrange("p (r b) -> p r b", r=RT),
                in_=s_d.rearrange("(r p) b -> p r b", p=128).bitcast(F32R),
            )
            s_sb = [s_all[:, RB * r : RB * (r + 1)] for r in range(RT)]
            blk = sb.tile([RB, CB], F32, name="blk", tag="blk", bufs=1)
            nc.vector.memset(blk[:], 0.0)

            # --- stage 1+2: stream X, reduce to [32, 32] ---
            for q in range(NQ):
                xts = []
                for r in range(RT):
                    xt = xp.tile([128, QW], F32R, name="xt", tag="xt", bufs=72)
                    nc.sync.dma_start(
                        out=xt[:],
                        in_=x_d[128 * r : 128 * (r + 1), q * QW : (q + 1) * QW].bitcast(F32R),
                    )
                    xts.append(xt)
                for j in range(NCH):
                    pt = ps.tile([RB, CH], F32, name="pt", tag="pt", bufs=6)
                    for r in range(RT):
                        lhs = s_sb[r]
                        rhs = xts[r][:, j * CH : (j + 1) * CH]
                        nc.tensor.matmul(
                            pt[:], lhs, rhs, start=(r == 0), stop=(r == RT - 1)
                        )
                    c0 = q * QW + j * CH
                    for cb in range(CB):
                        lo = max(int(col_cum[cb]), c0)
                        hi = min(int(col_cum[cb + 1]), c0 + CH)
                        if hi <= lo:
                            continue
                        tmp = sb.tile([RB, 1], F32, name="tmp", tag="tmp", bufs=4)
                        nc.vector.reduce_sum(
                            tmp[:], pt[:, lo - c0 : hi - c0], axis=mybir.AxisListType.X
                        )
                        nc.vector.tensor_add(
                            blk[:, cb : cb + 1], blk[:, cb : cb + 1], tmp[:]
                        )

            nc.sync.dma_start(out=blk_d, in_=blk[:])

            if DEVICE_MLP:
                # --- all-reduce the tiny [32, 32] partial sums ---
                cc_in = dp.tile([RB, CB], F32, name="cc_in", tag="cc_in", bufs=1)
                cc_out = dp.tile(
                    [RB, CB], F32, name="cc_out", tag="cc_out", bufs=1,
                    addr_space="Shared",
                )
                nc.sync.dma_start(out=cc_in[:], in_=blk[:])
                nc.gpsimd.collective_compute(
                    "AllReduce",
                    mybir.AluOpType.add,
                    replica_groups=[list(range(NCORES))],
                    ins=[cc_in.opt()],
                    outs=[cc_out.opt()],
                )

                # --- MLP weights ---
                w1_sb = []
                for k in range(8):
                    wt = sb.tile([128, HID], F32, name=f"w1_{k}", tag=f"w1_{k}", bufs=1)
                    nc.sync.dma_start(out=wt[:], in_=w1_d[128 * k : 128 * (k + 1), :])
                    w1_sb.append(wt)
                w2_sb = sb.tile([HID, HID], F32, name="w2s", tag="w2s", bufs=1)
                nc.sync.dma_start(out=w2_sb[:], in_=w2_d)
                w3_sb = sb.tile([HID, RB * CB], F32, name="w3s", tag="w3s", bufs=1)
                nc.sync.dma_start(out=w3_sb[:], in_=w3_d)
                b1_sb = sb.tile([HID, 1], F32, name="b1s", tag="b1s", bufs=1)
                nc.sync.dma_start(out=b1_sb[:], in_=b1_d)
                b2_sb = sb.tile([HID, 1], F32, name="b2s", tag="b2s", bufs=1)
                nc.sync.dma_start(out=b2_sb[:], in_=b2_d)
                b3_sb = sb.tile([128, 8], F32, name="b3s", tag="b3s", bufs=1)
                nc.sync.dma_start(out=b3_sb[:], in_=b3_d)
                ic_sb = sb.tile([128, 8], F32, name="ics", tag="ics", bufs=1)
                nc.sync.dma_start(out=ic_sb[:], in_=ic_d)

                # gather reduced sums into MLP input layout xcol[p, f] =
                # blk_red.flat[f*128 + p], then scale by 1/count
                xcol = sb.tile([128, 8], F32, name="xcol", tag="xcol", bufs=1)
                nc.sync.dma_start(
                    out=xcol[:],
                    in_=cc_out.rearrange("(f pp) q -> (pp q) f", pp=4),
                )
                nc.vector.tensor_mul(xcol[:], xcol[:], ic_sb[:])

                # layer 1: h1 = relu(W1.T @ x + b1) as [100, 1] column
                h1p = mps.tile([128, 1], F32, name="h1p", tag="mp", bufs=2)[0:HID, :]
                for k in range(8):
                    nc.tensor.matmul(
                        h1p[:], w1_sb[k][:], xcol[:, k : k + 1],
                        start=(k == 0), stop=(k == 7),
                    )
                h1 = sb.tile([HID, 1], F32, name="h1", tag="h1", bufs=1)
                nc.scalar.activation(
                    h1[:], h1p[:], mybir.ActivationFunctionType.Relu, bias=b1_sb[:]
                )
                # layer 2
                h2p = mps.tile([128, 1], F32, name="h2p", tag="mp", bufs=2)[0:HID, :]
                nc.tensor.matmul(h2p[:], w2_sb[:], h1[:], start=True, stop=True)
                h2 = sb.tile([HID, 1], F32, name="h2", tag="h2", bufs=1)
                nc.scalar.activation(
                    h2[:], h2p[:], mybir.ActivationFunctionType.Relu, bias=b2_sb[:]
                )
                # layer 3 + sigmoid, output as [128, 8] columns
                prop = sb.tile([128, 8], F32, name="prop_sb", tag="prop_sb", bufs=1)
                for k in range(8):
                    op = mps.tile([128, 1], F32, name="op", tag="mp", bufs=2)
                    nc.tensor.matmul(
                        op[:], w3_sb[:, 128 * k : 128 * (k + 1)], h2[:],
                        start=True, stop=True,
                    )
                    nc.scalar.activation(
                        prop[:, k : k + 1], op[:],
                        mybir.ActivationFunctionType.Sigmoid,
                        bias=b3_sb[:, k : k + 1],
                    )
                nc.sync.dma_start(out=prop_d, in_=prop[:])

    nc.compile()
    return nc


def kernel(X, row_ids, col_ids, W1, b1, W2, b2, W3, b3):
    from concourse.bass_utils import run_bass_kernel_spmd

    X = np.ascontiguousarray(np.asarray(X, dtype=np.float32))
    row_ids = np.asarray(row_ids, dtype=np.int32)
    col_ids = np.asarray(col_ids, dtype=np.int32)
    W1 = np.ascontiguousarray(np.asarray(W1, dtype=np.float32))
    W2 = np.ascontiguousarray(np.asarray(W2, dtype=np.float32))
    W3 = np.ascontiguousarray(np.asarray(W3, dtype=np.float32))
    b1 = np.asarray(b1, dtype=np.float32)
    b2 = np.asarray(b2, dtype=np.float32)
    b3 = np.asarray(b3, dtype=np.float32)

    rcnt = np.bincount(row_ids, minlength=RB).astype(np.int64)
    ccnt = np.bincount(col_ids, minlength=CB).astype(np.int64)
    row_cum = np.concatenate([[0], np.cumsum(rcnt)]).astype(np.int32)
    col_cum = np.concatenate([[0], np.cumsum(ccnt)]).astype(np.int32)

    key = col_cum.tobytes()
    if key not in _cache:
        _cache[key] = _build_program(col_cum)
    nc = _cache[key]

    # host-side index preprocessing
    S = np.zeros((N, RB), dtype=np.float32)
    S[np.arange(N), row_ids] = 1.0
    cnt = np.maximum(
        rcnt[:, None].astype(np.float32) * ccnt[None, :].astype(np.float32), 1.0
    )
    inv_cnt = (1.0 / cnt).astype(np.float32).reshape(-1)
    ic_col = np.ascontiguousarray(inv_cnt.reshape(8, 128).T)     # [128, 8]
    b3_col = np.ascontiguousarray(b3.reshape(8, 128).T)          # [128, 8]

    shared = {
        "w1": W1,
        "w2": W2,
        "w3": W3,
        "b1": np.ascontiguousarray(b1.reshape(HID, 1)),
        "b2": np.ascontiguousarray(b2.reshape(HID, 1)),
        "b3": b3_col,
        "ic": ic_col,
    }
    in_maps = []
    for c in range(NCORES):
        m = dict(shared)
        m["x"] = np.ascontiguousarray(X[c * RPC : (c + 1) * RPC, :])
        m["s"] = np.ascontiguousarray(S[c * RPC : (c + 1) * RPC, :])
        in_maps.append(m)

    t0 = time.perf_counter()
    try:
        res = run_bass_kernel_spmd(nc, in_maps, core_ids=list(range(NCORES)))
    except ModuleNotFoundError:
        # axon client without the NTFF profiling hook: force trace off
        os.environ["BASS_NEVER_TRACE"] = "1"
        res = run_bass_kernel_spmd(nc, in_maps, core_ids=list(range(NCORES)))
    t1 = time.perf_counter()
    LAST_PERF["exec_time_ns"] = res.exec_time_ns
    LAST_PERF["run_seconds"] = t1 - t0

    if DEVICE_MLP:
        prop_col = res.results[0]["prop"]                         # [128, 8]
        propensity = prop_col.T.reshape(-1).reshape(RB, CB).copy()
    else:
        blk = np.sum([r["blkpart"] for r in res.results], axis=0)
        x_small = (blk / cnt).reshape(-1)
        h = np.maximum(x_small @ W1 + b1, 0.0)
        h = np.maximum(h @ W2 + b2, 0.0)
        o = h @ W3 + b3
        propensity = (1.0 / (1.0 + np.exp(-o))).astype(np.float32).reshape(RB, CB)

    return propensity.astype(np.float32), row_cum, col_cum


# revision 12
# speedup vs baseline: 184738.8314x; 184738.8314x over previous
"""Trainium2 Bass kernel for MAR-block-missingness (segment_reduce).

Computes, for X [8192, 8192] f32 with sorted row_ids/col_ids in [0, 32):
  propensity = sigmoid(MLP(block_mean(X)))  [32, 32]
  row_cumsum, col_cumsum                    [33] int32 (index bookkeeping)

Strategy (8 NeuronCores, SPMD):
  - X row-sharded: core c gets rows [1024c, 1024(c+1)).
  - Stage 1 (PE): row-block segment sums via matmul with the one-hot
    row-block matrix S (host-built from row_ids): psum[32, 512] +=
    S_tile.T @ X_tile, accumulated over the 8 row tiles of the shard.
    float32r streaming mode for 1 cycle/row.
  - Stage 2 (DVE): column-block segment sums via reduce_sum over the
    sorted col-block ranges (compile-time baked from col_ids), added
    into a [32, 32] accumulator.
  - AllReduce the [32, 32] partial block sums across the 8 cores.
  - MLP (replicated on every core): x = blk * inv_cnt laid out as
    [128, 8] columns; h1 = relu(W1.T x + b1) as [100, 1] column;
    h2 = relu(W2.T h1 + b2); out = sigmoid(W3.T h2 + b3) as [128, 8].
"""

import os
import time

import numpy as np

N = 8192
T = 8192
RB = 32
CB = 32
HID = 100
NCORES = 8
RPC = N // NCORES      # rows per core (1024)
RT = RPC // 128        # 128-row tiles per core (8)
QW = 512               # column unit width (0.25 MB DMA tiles)
NQ = T // QW           # 4
CH = 512               # psum chunk width (one PSUM bank of f32)
NCH = QW // CH         # 4

USE_F32R = True        # float32r streaming matmul (4x faster than f32)
DEVICE_MLP = True      # all-reduce + MLP on device; else host finishes

_cache = {}
LAST_PERF = {"exec_time_ns": None, "run_seconds": None}


def _build_program(col_cum):
    """Build + compile the SPMD Bass program. col_cum: [33] col-block
    boundaries (cumsum of col bincounts), baked in as compile-time
    reduce ranges."""
    import concourse.bacc as bacc
    import concourse.mybir as mybir
    import concourse.tile as tile

    F32 = mybir.dt.float32
    F32R = mybir.dt.float32r

    nc = bacc.Bacc(
        "TRN2",
        target_bir_lowering=False,
        debug=False,
        enable_asserts=False,
        num_devices=NCORES,
    )

    x_d = nc.dram_tensor("x", [RPC, T], F32, kind="ExternalInput").ap()
    s_d = nc.dram_tensor("s", [RPC, RB], F32, kind="ExternalInput").ap()
    w1_d = nc.dram_tensor("w1", [RB, CB * HID], F32, kind="ExternalInput").ap()
    w2_d = nc.dram_tensor("w2", [HID, HID], F32, kind="ExternalInput").ap()
    w3_d = nc.dram_tensor("w3", [HID, RB * CB], F32, kind="ExternalInput").ap()
    b1_d = nc.dram_tensor("b1", [HID, 1], F32, kind="ExternalInput").ap()
    b2_d = nc.dram_tensor("b2", [HID, 1], F32, kind="ExternalInput").ap()
    b3_d = nc.dram_tensor("b3", [128, 8], F32, kind="ExternalInput").ap()
    ic_d = nc.dram_tensor("ic", [RB, CB], F32, kind="ExternalInput").ap()
    blk_d = nc.dram_tensor("blkpart", [RB, CB], F32, kind="ExternalOutput").ap()
    if DEVICE_MLP:
        prop_d = nc.dram_tensor("prop", [128, 8], F32, kind="ExternalOutput").ap()

    with tile.TileContext(nc) as tc:
        with (
            tc.tile_pool(name="sb", bufs=1) as sb,
            tc.tile_pool(name="xp", bufs=16) as xp,
            tc.tile_pool(name="ps", bufs=6, space="PSUM") as ps,
            tc.tile_pool(name="mps", bufs=2, space="PSUM") as mps,
            tc.tile_pool(name="dp", bufs=1, space="DRAM") as dp,
        ):
            # --- constants: one-hot row-block tiles (one DMA, side by side) ---
            s_all = sb.tile([128, RT * RB], F32R, name="s_all", tag="s_all", bufs=1)
            nc.sync.dma_start(
                out=s_all[:].rearrange("p (r b) -> p r b", r=RT),
                in_=s_d.rearrange("(r p) b -> p r b", p=128).bitcast(F32R),
            )
            s_sb = [s_all[:, RB * r : RB * (r + 1)] for r in range(RT)]
            blk = sb.tile([RB, CB], F32, name="blk", tag="blk", bufs=1)
            nc.vector.memset(blk[:], 0.0)

            if DEVICE_MLP:
                # MLP weights: load up-front on the ACT HWDGE ring so they
                # don't queue behind the X stream on the SP ring
                w1_sb = sb.tile([RB, CB * HID], F32, name="w1s", tag="w1s", bufs=1)
                nc.scalar.dma_start(out=w1_sb[:], in_=w1_d)
                w2_sb = sb.tile([HID, HID], F32, name="w2s", tag="w2s", bufs=1)
                nc.scalar.dma_start(out=w2_sb[:], in_=w2_d)
                w3_sb = sb.tile([HID, RB * CB], F32, name="w3s", tag="w3s", bufs=1)
                nc.scalar.dma_start(out=w3_sb[:], in_=w3_d)
                b1_sb = sb.tile([HID, 1], F32, name="b1s", tag="b1s", bufs=1)
                nc.scalar.dma_start(out=b1_sb[:], in_=b1_d)
                b2_sb = sb.tile([HID, 1], F32, name="b2s", tag="b2s", bufs=1)
                nc.scalar.dma_start(out=b2_sb[:], in_=b2_d)
                b3_sb = sb.tile([128, 8], F32, name="b3s", tag="b3s", bufs=1)
                nc.scalar.dma_start(out=b3_sb[:], in_=b3_d)
                ic_sb = sb.tile([RB, CB], F32, name="ics", tag="ics", bufs=1)
                nc.scalar.dma_start(out=ic_sb[:], in_=ic_d)
                # prewarm ACT LUTs for Relu/Sigmoid so the tail doesn't pay
                # the cold table load
                warm = sb.tile([1, 1], F32, name="warm", tag="warm", bufs=1)
                nc.vector.memset(warm[:], 0.0)
                nc.scalar.activation(warm[:], warm[:], mybir.ActivationFunctionType.Relu)
                nc.scalar.activation(warm[:], warm[:], mybir.ActivationFunctionType.Sigmoid)

            # --- stage 1+2: stream X, reduce to [32, 32] ---
            for q in range(NQ):
                xts = []
                for r in range(RT):
                    xt = xp.tile([128, QW], F32R, name="xt", tag="xt", bufs=72)
                    nc.sync.dma_start(
                        out=xt[:],
                        in_=x_d[128 * r : 128 * (r + 1), q * QW : (q + 1) * QW].bitcast(F32R),
                    )
                    xts.append(xt)
                for j in range(NCH):
                    pt = ps.tile([RB, CH], F32, name="pt", tag="pt", bufs=6)
                    for r in range(RT):
                        lhs = s_sb[r]
                        rhs = xts[r][:, j * CH : (j + 1) * CH]
                        nc.tensor.matmul(
                            pt[:], lhs, rhs, start=(r == 0), stop=(r == RT - 1)
                        )
                    c0 = q * QW + j * CH
                    for cb in range(CB):
                        lo = max(int(col_cum[cb]), c0)
                        hi = min(int(col_cum[cb + 1]), c0 + CH)
                        if hi <= lo:
                            continue
                        tmp = sb.tile([RB, 1], F32, name="tmp", tag="tmp", bufs=4)
                        nc.vector.reduce_sum(
                            tmp[:], pt[:, lo - c0 : hi - c0], axis=mybir.AxisListType.X
                        )
                        nc.vector.tensor_add(
                            blk[:, cb : cb + 1], blk[:, cb : cb + 1], tmp[:]
                        )

            if DEVICE_MLP:
                # --- all-gather the tiny [32, 32] partial sums ---
                cc_in = dp.tile([RB, CB], F32, name="cc_in", tag="cc_in", bufs=1)
                cc_gat = dp.tile(
                    [NCORES * RB, CB], F32, name="cc_gat", tag="cc_gat", bufs=1,
                    addr_space="Shared",
                )
                nc.sync.dma_start(out=cc_in[:], in_=blk[:])
                nc.scalar.dma_start(out=blk_d, in_=blk[:])
                nc.gpsimd.collective_compute(
                    "AllGather",
                    mybir.AluOpType.bypass,
                    replica_groups=[list(range(NCORES))],
                    ins=[cc_in.opt()],
                    outs=[cc_gat.opt()],
                )

                # gather all 8 partials [32, (c cb)] with 128B-contiguous runs
                xg = sb.tile([RB, NCORES * CB], F32, name="xg", tag="xg", bufs=1)
                nc.sync.dma_start(
                    out=xg[:].rearrange("p (c q) -> p c q", c=NCORES),
                    in_=cc_gat.rearrange("(c p) q -> p c q", p=RB),
                )
                xs = sb.tile([RB, CB], F32, name="xs", tag="xs", bufs=1)
                nc.vector.tensor_add(xs[:], xg[:, 0:CB], xg[:, CB : 2 * CB])
                for c in range(2, NCORES):
                    nc.vector.tensor_add(
                        xs[:], xs[:], xg[:, CB * c : CB * (c + 1)]
                    )
                nc.vector.tensor_mul(xs[:], xs[:], ic_sb[:])

                # layer 1: h1 = relu(W1.T @ x + b1) as [100, 1] column;
                # contract over rb in 32 cb-chunks: lhsT = W1[rb, cb, :]
                h1p = mps.tile([128, 1], F32, name="h1p", tag="mp", bufs=2)[0:HID, :]
                for cb in range(CB):
                    nc.tensor.matmul(
                        h1p[:], w1_sb[:, HID * cb : HID * (cb + 1)],
                        xs[:, cb : cb + 1],
                        start=(cb == 0), stop=(cb == CB - 1),
                    )
                h1 = sb.tile([HID, 1], F32, name="h1", tag="h1", bufs=1)
                nc.scalar.activation(
                    h1[:], h1p[:], mybir.ActivationFunctionType.Relu, bias=b1_sb[:]
                )
                # layer 2
                h2p = mps.tile([128, 1], F32, name="h2p", tag="mp", bufs=2)[0:HID, :]
                nc.tensor.matmul(h2p[:], w2_sb[:], h1[:], start=True, stop=True)
                h2 = sb.tile([HID, 1], F32, name="h2", tag="h2", bufs=1)
                nc.scalar.activation(
                    h2[:], h2p[:], mybir.ActivationFunctionType.Relu, bias=b2_sb[:]
                )
                # layer 3 + sigmoid, output as [128, 8] columns
                prop = sb.tile([128, 8], F32, name="prop_sb", tag="prop_sb", bufs=1)
                for k in range(8):
                    op = mps.tile([128, 1], F32, name="op", tag="mp", bufs=2)
                    nc.tensor.matmul(
                        op[:], w3_sb[:, 128 * k : 128 * (k + 1)], h2[:],
                        start=True, stop=True,
                    )
                    nc.scalar.activation(
                        prop[:, k : k + 1], op[:],
                        mybir.ActivationFunctionType.Sigmoid,
                        bias=b3_sb[:, k : k + 1],
                    )
                nc.sync.dma_start(out=prop_d, in_=prop[:])
            else:
                nc.sync.dma_start(out=blk_d, in_=blk[:])

    nc.compile()
    return nc


def kernel(X, row_ids, col_ids, W1, b1, W2, b2, W3, b3):
    from concourse.bass_utils import run_bass_kernel_spmd

    X = np.ascontiguousarray(np.asarray(X, dtype=np.float32))
    row_ids = np.asarray(row_ids, dtype=np.int32)
    col_ids = np.asarray(col_ids, dtype=np.int32)
    W1 = np.ascontiguousarray(np.asarray(W1, dtype=np.float32))
    W2 = np.ascontiguousarray(np.asarray(W2, dtype=np.float32))
    W3 = np.ascontiguousarray(np.asarray(W3, dtype=np.float32))
    b1 = np.asarray(b1, dtype=np.float32)
    b2 = np.asarray(b2, dtype=np.float32)
    b3 = np.asarray(b3, dtype=np.float32)

    rcnt = np.bincount(row_ids, minlength=RB).astype(np.int64)
    ccnt = np.bincount(col_ids, minlength=CB).astype(np.int64)
    row_cum = np.concatenate([[0], np.cumsum(rcnt)]).astype(np.int32)
    col_cum = np.concatenate([[0], np.cumsum(ccnt)]).astype(np.int32)

    key = col_cum.tobytes()
    if key not in _cache:
        _cache[key] = _build_program(col_cum)
    nc = _cache[key]

    # host-side index preprocessing
    S = np.zeros((N, RB), dtype=np.float32)
    S[np.arange(N), row_ids] = 1.0
    cnt = np.maximum(
        rcnt[:, None].astype(np.float32) * ccnt[None, :].astype(np.float32), 1.0
    )
    ic_rc = np.ascontiguousarray((1.0 / cnt).astype(np.float32))  # [32, 32]
    b3_col = np.ascontiguousarray(b3.reshape(8, 128).T)          # [128, 8]

    w1r = np.ascontiguousarray(W1.reshape(RB, CB * HID))
    shared = {
        "w1": w1r,
        "w2": W2,
        "w3": W3,
        "b1": np.ascontiguousarray(b1.reshape(HID, 1)),
        "b2": np.ascontiguousarray(b2.reshape(HID, 1)),
        "b3": b3_col,
        "ic": ic_rc,
    }
    in_maps = []
    for c in range(NCORES):
        m = dict(shared)
        m["x"] = np.ascontiguousarray(X[c * RPC : (c + 1) * RPC, :])
        m["s"] = np.ascontiguousarray(S[c * RPC : (c + 1) * RPC, :])
        in_maps.append(m)

    t0 = time.perf_counter()
    try:
        res = run_bass_kernel_spmd(nc, in_maps, core_ids=list(range(NCORES)))
    except ModuleNotFoundError:
        # axon client without the NTFF profiling hook: force trace off
        os.environ["BASS_NEVER_TRACE"] = "1"
        res = run_bass_kernel_spmd(nc, in_maps, core_ids=list(range(NCORES)))
    t1 = time.perf_counter()
    LAST_PERF["exec_time_ns"] = res.exec_time_ns
    LAST_PERF["run_seconds"] = t1 - t0

    if DEVICE_MLP:
        prop_col = res.results[0]["prop"]                         # [128, 8]
        propensity = prop_col.T.reshape(-1).reshape(RB, CB).copy()
    else:
        blk = np.sum([r["blkpart"] for r in res.results], axis=0)
        x_small = (blk / cnt).reshape(-1)
        h = np.maximum(x_small @ W1 + b1, 0.0)
        h = np.maximum(h @ W2 + b2, 0.0)
        o = h @ W3 + b3
        propensity = (1.0 / (1.0 + np.exp(-o))).astype(np.float32).reshape(RB, CB)

    return propensity.astype(np.float32), row_cum, col_cum


# revision 13
# speedup vs baseline: 188299.4483x; 1.0193x over previous
"""Trainium2 Bass kernel for MAR-block-missingness (segment_reduce).

Computes, for X [8192, 8192] f32 with sorted row_ids/col_ids in [0, 32):
  propensity = sigmoid(MLP(block_mean(X)))  [32, 32]
  row_cumsum, col_cumsum                    [33] int32 (index bookkeeping)

Strategy (8 NeuronCores, SPMD):
  - X row-sharded: core c gets rows [1024c, 1024(c+1)).
  - Stage 1 (PE): row-block segment sums via matmul with the one-hot
    row-block matrix S (host-built from row_ids): psum[32, 512] +=
    S_tile.T @ X_tile, accumulated over the 8 row tiles of the shard.
    float32r streaming mode for 1 cycle/row.
  - Stage 2 (DVE): column-block segment sums via reduce_sum over the
    sorted col-block ranges (compile-time baked from col_ids), added
    into a [32, 32] accumulator.
  - AllReduce the [32, 32] partial block sums across the 8 cores.
  - MLP (replicated on every core): x = blk * inv_cnt laid out as
    [128, 8] columns; h1 = relu(W1.T x + b1) as [100, 1] column;
    h2 = relu(W2.T h1 + b2); out = sigmoid(W3.T h2 + b3) as [128, 8].
"""

import os
import time

import numpy as np

N = 8192
T = 8192
RB = 32
CB = 32
HID = 100
NCORES = 8
RPC = N // NCORES      # rows per core (1024)
RT = RPC // 128        # 128-row tiles per core (8)
QW = 512               # column unit width (0.25 MB DMA tiles)
NQ = T // QW           # 4
CH = 512               # psum chunk width (one PSUM bank of f32)
NCH = QW // CH         # 4

USE_F32R = True        # float32r streaming matmul (4x faster than f32)
DEVICE_MLP = True      # all-reduce + MLP on device; else host finishes

_cache = {}
LAST_PERF = {"exec_time_ns": None, "run_seconds": None}


def _build_program(col_cum):
    """Build + compile the SPMD Bass program. col_cum: [33] col-block
    boundaries (cumsum of col bincounts), baked in as compile-time
    reduce ranges."""
    import concourse.bacc as bacc
    import concourse.mybir as mybir
    import concourse.tile as tile

    F32 = mybir.dt.float32
    F32R = mybir.dt.float32r

    nc = bacc.Bacc(
        "TRN2",
        target_bir_lowering=False,
        debug=False,
        enable_asserts=False,
        num_devices=NCORES,
    )

    x_d = nc.dram_tensor("x", [RPC, T], F32, kind="ExternalInput").ap()
    s_d = nc.dram_tensor("s", [RPC, RB], F32, kind="ExternalInput").ap()
    w1_d = nc.dram_tensor("w1", [RB, CB * HID], F32, kind="ExternalInput").ap()
    w2_d = nc.dram_tensor("w2", [HID, HID], F32, kind="ExternalInput").ap()
    w3_d = nc.dram_tensor("w3", [HID, RB * CB], F32, kind="ExternalInput").ap()
    b1_d = nc.dram_tensor("b1", [HID, 1], F32, kind="ExternalInput").ap()
    b2_d = nc.dram_tensor("b2", [HID, 1], F32, kind="ExternalInput").ap()
    b3_d = nc.dram_tensor("b3", [128, 8], F32, kind="ExternalInput").ap()
    ic_d = nc.dram_tensor("ic", [RB, CB], F32, kind="ExternalInput").ap()
    blk_d = nc.dram_tensor("blkpart", [RB, CB], F32, kind="ExternalOutput").ap()
    if DEVICE_MLP:
        prop_d = nc.dram_tensor("prop", [128, 8], F32, kind="ExternalOutput").ap()

    with tile.TileContext(nc) as tc:
        with (
            tc.tile_pool(name="sb", bufs=1) as sb,
            tc.tile_pool(name="xp", bufs=16) as xp,
            tc.tile_pool(name="ps", bufs=6, space="PSUM") as ps,
            tc.tile_pool(name="mps", bufs=2, space="PSUM") as mps,
            tc.tile_pool(name="dp", bufs=1, space="DRAM") as dp,
        ):
            # --- constants: one-hot row-block tiles (one DMA, side by side) ---
            s_all = sb.tile([128, RT * RB], F32R, name="s_all", tag="s_all", bufs=1)
            nc.sync.dma_start(
                out=s_all[:].rearrange("p (r b) -> p r b", r=RT),
                in_=s_d.rearrange("(r p) b -> p r b", p=128).bitcast(F32R),
            )
            s_sb = [s_all[:, RB * r : RB * (r + 1)] for r in range(RT)]
            blk = sb.tile([RB, CB], F32, name="blk", tag="blk", bufs=1)
            nc.vector.memset(blk[:], 0.0)

            if DEVICE_MLP:
                # prewarm ACT LUTs for Relu/Sigmoid so the tail doesn't pay
                # the cold table load
                warm = sb.tile([1, 1], F32, name="warm", tag="warm", bufs=1)
                nc.vector.memset(warm[:], 0.0)
                nc.scalar.activation(warm[:], warm[:], mybir.ActivationFunctionType.Relu)
                nc.scalar.activation(warm[:], warm[:], mybir.ActivationFunctionType.Sigmoid)

            # --- stage 1+2: stream X, reduce to [32, 32] ---
            for q in range(NQ):
                xts = []
                for r in range(RT):
                    xt = xp.tile([128, QW], F32R, name="xt", tag="xt", bufs=72)
                    last_x_dma = nc.sync.dma_start(
                        out=xt[:],
                        in_=x_d[128 * r : 128 * (r + 1), q * QW : (q + 1) * QW].bitcast(F32R),
                    )
                    xts.append(xt)
                for j in range(NCH):
                    pt = ps.tile([RB, CH], F32, name="pt", tag="pt", bufs=6)
                    for r in range(RT):
                        lhs = s_sb[r]
                        rhs = xts[r][:, j * CH : (j + 1) * CH]
                        nc.tensor.matmul(
                            pt[:], lhs, rhs, start=(r == 0), stop=(r == RT - 1)
                        )
                    c0 = q * QW + j * CH
                    for cb in range(CB):
                        lo = max(int(col_cum[cb]), c0)
                        hi = min(int(col_cum[cb + 1]), c0 + CH)
                        if hi <= lo:
                            continue
                        tmp = sb.tile([RB, 1], F32, name="tmp", tag="tmp", bufs=4)
                        nc.vector.reduce_sum(
                            tmp[:], pt[:, lo - c0 : hi - c0], axis=mybir.AxisListType.X
                        )
                        nc.vector.tensor_add(
                            blk[:, cb : cb + 1], blk[:, cb : cb + 1], tmp[:]
                        )

            if DEVICE_MLP:
                # MLP weights: loaded on the ACT HWDGE ring, explicitly
                # delayed past the last X-tile DMA so their bytes ride the
                # collective's dead window instead of the saturated stream
                from concourse.tile import add_dep_helper

                w1_sb = sb.tile([RB, CB * HID], F32, name="w1s", tag="w1s", bufs=1)
                w_dmas = [nc.scalar.dma_start(out=w1_sb[:], in_=w1_d)]
                w2_sb = sb.tile([HID, HID], F32, name="w2s", tag="w2s", bufs=1)
                w_dmas.append(nc.scalar.dma_start(out=w2_sb[:], in_=w2_d))
                w3_sb = sb.tile([HID, RB * CB], F32, name="w3s", tag="w3s", bufs=1)
                w_dmas.append(nc.scalar.dma_start(out=w3_sb[:], in_=w3_d))
                b1_sb = sb.tile([HID, 1], F32, name="b1s", tag="b1s", bufs=1)
                w_dmas.append(nc.scalar.dma_start(out=b1_sb[:], in_=b1_d))
                b2_sb = sb.tile([HID, 1], F32, name="b2s", tag="b2s", bufs=1)
                w_dmas.append(nc.scalar.dma_start(out=b2_sb[:], in_=b2_d))
                b3_sb = sb.tile([128, 8], F32, name="b3s", tag="b3s", bufs=1)
                w_dmas.append(nc.scalar.dma_start(out=b3_sb[:], in_=b3_d))
                ic_sb = sb.tile([RB, CB], F32, name="ics", tag="ics", bufs=1)
                w_dmas.append(nc.scalar.dma_start(out=ic_sb[:], in_=ic_d))
                for wd in w_dmas:
                    add_dep_helper(
                        wd.ins, last_x_dma.ins, sync=True,
                        reason="weight loads after X stream",
                    )

                # --- all-gather the tiny [32, 32] partial sums ---
                cc_in = dp.tile([RB, CB], F32, name="cc_in", tag="cc_in", bufs=1)
                cc_gat = dp.tile(
                    [NCORES * RB, CB], F32, name="cc_gat", tag="cc_gat", bufs=1,
                    addr_space="Shared",
                )
                nc.sync.dma_start(out=cc_in[:], in_=blk[:])
                nc.scalar.dma_start(out=blk_d, in_=blk[:])
                nc.gpsimd.collective_compute(
                    "AllGather",
                    mybir.AluOpType.bypass,
                    replica_groups=[list(range(NCORES))],
                    ins=[cc_in.opt()],
                    outs=[cc_gat.opt()],
                )

                # gather all 8 partials [32, (c cb)] with 128B-contiguous runs
                xg = sb.tile([RB, NCORES * CB], F32, name="xg", tag="xg", bufs=1)
                nc.sync.dma_start(
                    out=xg[:].rearrange("p (c q) -> p c q", c=NCORES),
                    in_=cc_gat.rearrange("(c p) q -> p c q", p=RB),
                )
                xs = sb.tile([RB, CB], F32, name="xs", tag="xs", bufs=1)
                nc.vector.tensor_add(xs[:], xg[:, 0:CB], xg[:, CB : 2 * CB])
                for c in range(2, NCORES):
                    nc.vector.tensor_add(
                        xs[:], xs[:], xg[:, CB * c : CB * (c + 1)]
                    )
                nc.vector.tensor_mul(xs[:], xs[:], ic_sb[:])

                # layer 1: h1 = relu(W1.T @ x + b1) as [100, 1] column;
                # contract over rb in 32 cb-chunks: lhsT = W1[rb, cb, :]
                h1p = mps.tile([128, 1], F32, name="h1p", tag="mp", bufs=2)[0:HID, :]
                for cb in range(CB):
                    nc.tensor.matmul(
                        h1p[:], w1_sb[:, HID * cb : HID * (cb + 1)],
                        xs[:, cb : cb + 1],
                        start=(cb == 0), stop=(cb == CB - 1),
                    )
                h1 = sb.tile([HID, 1], F32, name="h1", tag="h1", bufs=1)
                nc.scalar.activation(
                    h1[:], h1p[:], mybir.ActivationFunctionType.Relu, bias=b1_sb[:]
                )
                # layer 2
                h2p = mps.tile([128, 1], F32, name="h2p", tag="mp", bufs=2)[0:HID, :]
                nc.tensor.matmul(h2p[:], w2_sb[:], h1[:], start=True, stop=True)
                h2 = sb.tile([HID, 1], F32, name="h2", tag="h2", bufs=1)
                nc.scalar.activation(
                    h2[:], h2p[:], mybir.ActivationFunctionType.Relu, bias=b2_sb[:]
                )
                # layer 3 + sigmoid, output as [128, 8] columns
                prop = sb.tile([128, 8], F32, name="prop_sb", tag="prop_sb", bufs=1)
                for k in range(8):
                    op = mps.tile([128, 1], F32, name="op", tag="mp", bufs=2)
                    nc.tensor.matmul(
                        op[:], w3_sb[:, 128 * k : 128 * (k + 1)], h2[:],
                        start=True, stop=True,
                    )
                    nc.scalar.activation(
                        prop[:, k : k + 1], op[:],
                        mybir.ActivationFunctionType.Sigmoid,
                        bias=b3_sb[:, k : k + 1],
                    )
                nc.sync.dma_start(out=prop_d, in_=prop[:])
            else:
                nc.sync.dma_start(out=blk_d, in_=blk[:])

    nc.compile()
    return nc


def kernel(X, row_ids, col_ids, W1, b1, W2, b2, W3, b3):
    from concourse.bass_utils import run_bass_kernel_spmd

    X = np.ascontiguousarray(np.asarray(X, dtype=np.float32))
    row_ids = np.asarray(row_ids, dtype=np.int32)
    col_ids = np.asarray(col_ids, dtype=np.int32)
    W1 = np.ascontiguousarray(np.asarray(W1, dtype=np.float32))
    W2 = np.ascontiguousarray(np.asarray(W2, dtype=np.float32))
    W3 = np.ascontiguousarray(np.asarray(W3, dtype=np.float32))
    b1 = np.asarray(b1, dtype=np.float32)
    b2 = np.asarray(b2, dtype=np.float32)
    b3 = np.asarray(b3, dtype=np.float32)

    rcnt = np.bincount(row_ids, minlength=RB).astype(np.int64)
    ccnt = np.bincount(col_ids, minlength=CB).astype(np.int64)
    row_cum = np.concatenate([[0], np.cumsum(rcnt)]).astype(np.int32)
    col_cum = np.concatenate([[0], np.cumsum(ccnt)]).astype(np.int32)

    key = col_cum.tobytes()
    if key not in _cache:
        _cache[key] = _build_program(col_cum)
    nc = _cache[key]

    # host-side index preprocessing
    S = np.zeros((N, RB), dtype=np.float32)
    S[np.arange(N), row_ids] = 1.0
    cnt = np.maximum(
        rcnt[:, None].astype(np.float32) * ccnt[None, :].astype(np.float32), 1.0
    )
    ic_rc = np.ascontiguousarray((1.0 / cnt).astype(np.float32))  # [32, 32]
    b3_col = np.ascontiguousarray(b3.reshape(8, 128).T)          # [128, 8]

    w1r = np.ascontiguousarray(W1.reshape(RB, CB * HID))
    shared = {
        "w1": w1r,
        "w2": W2,
        "w3": W3,
        "b1": np.ascontiguousarray(b1.reshape(HID, 1)),
        "b2": np.ascontiguousarray(b2.reshape(HID, 1)),
        "b3": b3_col,
        "ic": ic_rc,
    }
    in_maps = []
    for c in range(NCORES):
        m = dict(shared)
        m["x"] = np.ascontiguousarray(X[c * RPC : (c + 1) * RPC, :])
        m["s"] = np.ascontiguousarray(S[c * RPC : (c + 1) * RPC, :])
        in_maps.append(m)

    t0 = time.perf_counter()
    try:
        res = run_bass_kernel_spmd(nc, in_maps, core_ids=list(range(NCORES)))
    except ModuleNotFoundError:
        # axon client without the NTFF profiling hook: force trace off
        os.environ["BASS_NEVER_TRACE"] = "1"
        res = run_bass_kernel_spmd(nc, in_maps, core_ids=list(range(NCORES)))
    t1 = time.perf_counter()
    LAST_PERF["exec_time_ns"] = res.exec_time_ns
    LAST_PERF["run_seconds"] = t1 - t0

    if DEVICE_MLP:
        prop_col = res.results[0]["prop"]                         # [128, 8]
        propensity = prop_col.T.reshape(-1).reshape(RB, CB).copy()
    else:
        blk = np.sum([r["blkpart"] for r in res.results], axis=0)
        x_small = (blk / cnt).reshape(-1)
        h = np.maximum(x_small @ W1 + b1, 0.0)
        h = np.maximum(h @ W2 + b2, 0.0)
        o = h @ W3 + b3
        propensity = (1.0 / (1.0 + np.exp(-o))).astype(np.float32).reshape(RB, CB)

    return propensity.astype(np.float32), row_cum, col_cum


# revision 14
# speedup vs baseline: 189690.4519x; 1.0074x over previous
"""Trainium2 Bass kernel for MAR-block-missingness (segment_reduce).

Computes, for X [8192, 8192] f32 with sorted row_ids/col_ids in [0, 32):
  propensity = sigmoid(MLP(block_mean(X)))  [32, 32]
  row_cumsum, col_cumsum                    [33] int32 (index bookkeeping)

Strategy (8 NeuronCores, SPMD):
  - X row-sharded: core c gets rows [1024c, 1024(c+1)).
  - Stage 1 (PE): row-block segment sums via matmul with the one-hot
    row-block matrix S (host-built from row_ids): psum[32, 512] +=
    S_tile.T @ X_tile, accumulated over the 8 row tiles of the shard.
    float32r streaming mode for 1 cycle/row.
  - Stage 2 (DVE): column-block segment sums via reduce_sum over the
    sorted col-block ranges (compile-time baked from col_ids), added
    into a [32, 32] accumulator.
  - AllReduce the [32, 32] partial block sums across the 8 cores.
  - MLP (replicated on every core): x = blk * inv_cnt laid out as
    [128, 8] columns; h1 = relu(W1.T x + b1) as [100, 1] column;
    h2 = relu(W2.T h1 + b2); out = sigmoid(W3.T h2 + b3) as [128, 8].
"""

import os
import time

import numpy as np

N = 8192
T = 8192
RB = 32
CB = 32
HID = 100
NCORES = 8
RPC = N // NCORES      # rows per core (1024)
RT = RPC // 128        # 128-row tiles per core (8)
QW = 512               # column unit width (0.25 MB DMA tiles)
NQ = T // QW           # 4
CH = 512               # psum chunk width (one PSUM bank of f32)
NCH = QW // CH         # 4

USE_F32R = True        # float32r streaming matmul (4x faster than f32)
DEVICE_MLP = True      # all-reduce + MLP on device; else host finishes

_cache = {}
LAST_PERF = {"exec_time_ns": None, "run_seconds": None}


def _build_program(col_cum):
    """Build + compile the SPMD Bass program. col_cum: [33] col-block
    boundaries (cumsum of col bincounts), baked in as compile-time
    reduce ranges."""
    import concourse.bacc as bacc
    import concourse.mybir as mybir
    import concourse.tile as tile

    F32 = mybir.dt.float32
    F32R = mybir.dt.float32r

    nc = bacc.Bacc(
        "TRN2",
        target_bir_lowering=False,
        debug=False,
        enable_asserts=False,
        num_devices=NCORES,
    )

    x_d = nc.dram_tensor("x", [RPC, T], F32, kind="ExternalInput").ap()
    s_d = nc.dram_tensor("s", [RPC, RB], F32, kind="ExternalInput").ap()
    w1_d = nc.dram_tensor("w1", [RB, CB * HID], F32, kind="ExternalInput").ap()
    w2_d = nc.dram_tensor("w2", [HID, HID], F32, kind="ExternalInput").ap()
    w3_d = nc.dram_tensor("w3", [HID, RB * CB], F32, kind="ExternalInput").ap()
    b1_d = nc.dram_tensor("b1", [HID, 1], F32, kind="ExternalInput").ap()
    b2_d = nc.dram_tensor("b2", [HID, 1], F32, kind="ExternalInput").ap()
    b3_d = nc.dram_tensor("b3", [128, 8], F32, kind="ExternalInput").ap()
    ic_d = nc.dram_tensor("ic", [RB, CB], F32, kind="ExternalInput").ap()
    blk_d = nc.dram_tensor("blkpart", [RB, CB], F32, kind="ExternalOutput").ap()
    if DEVICE_MLP:
        prop_d = nc.dram_tensor("prop", [128, 8], F32, kind="ExternalOutput").ap()

    with tile.TileContext(nc) as tc:
        with (
            tc.tile_pool(name="sb", bufs=1) as sb,
            tc.tile_pool(name="xp", bufs=16) as xp,
            tc.tile_pool(name="ps", bufs=6, space="PSUM") as ps,
            tc.tile_pool(name="mps", bufs=2, space="PSUM") as mps,
            tc.tile_pool(name="dp", bufs=1, space="DRAM") as dp,
        ):
            # --- constants: one-hot row-block tiles (one DMA, side by side) ---
            s_all = sb.tile([128, RT * RB], F32R, name="s_all", tag="s_all", bufs=1)
            nc.sync.dma_start(
                out=s_all[:].rearrange("p (r b) -> p r b", r=RT),
                in_=s_d.rearrange("(r p) b -> p r b", p=128).bitcast(F32R),
            )
            s_sb = [s_all[:, RB * r : RB * (r + 1)] for r in range(RT)]
            blk = sb.tile([RB, CB], F32, name="blk", tag="blk", bufs=1)
            nc.vector.memset(blk[:], 0.0)

            if DEVICE_MLP:
                # prewarm ACT LUTs for Relu/Sigmoid so the tail doesn't pay
                # the cold table load
                warm = sb.tile([1, 1], F32, name="warm", tag="warm", bufs=1)
                nc.vector.memset(warm[:], 0.0)
                nc.scalar.activation(warm[:], warm[:], mybir.ActivationFunctionType.Relu)
                nc.scalar.activation(warm[:], warm[:], mybir.ActivationFunctionType.Sigmoid)

            # --- stage 1+2: stream X, reduce to [32, 32] ---
            for q in range(NQ):
                xts = []
                for r in range(RT):
                    xt = xp.tile([128, QW], F32R, name="xt", tag="xt", bufs=72)
                    last_x_dma = nc.sync.dma_start(
                        out=xt[:],
                        in_=x_d[128 * r : 128 * (r + 1), q * QW : (q + 1) * QW].bitcast(F32R),
                    )
                    xts.append(xt)
                for j in range(NCH):
                    pt = ps.tile([RB, CH], F32, name="pt", tag="pt", bufs=6)
                    for r in range(RT):
                        lhs = s_sb[r]
                        rhs = xts[r][:, j * CH : (j + 1) * CH]
                        nc.tensor.matmul(
                            pt[:], lhs, rhs, start=(r == 0), stop=(r == RT - 1)
                        )
                    c0 = q * QW + j * CH
                    for cb in range(CB):
                        lo = max(int(col_cum[cb]), c0)
                        hi = min(int(col_cum[cb + 1]), c0 + CH)
                        if hi <= lo:
                            continue
                        tmp = sb.tile([RB, 1], F32, name="tmp", tag="tmp", bufs=4)
                        nc.vector.reduce_sum(
                            tmp[:], pt[:, lo - c0 : hi - c0], axis=mybir.AxisListType.X
                        )
                        nc.vector.tensor_add(
                            blk[:, cb : cb + 1], blk[:, cb : cb + 1], tmp[:]
                        )

            if DEVICE_MLP:
                from concourse.tile import add_dep_helper

                # --- all-gather the tiny [32, 32] partial sums ---
                cc_in = dp.tile([RB, CB], F32, name="cc_in", tag="cc_in", bufs=1)
                cc_gat = dp.tile(
                    [NCORES * RB, CB], F32, name="cc_gat", tag="cc_gat", bufs=1,
                    addr_space="Shared",
                )
                cc_dma = nc.sync.dma_start(out=cc_in[:], in_=blk[:])
                nc.gpsimd.collective_compute(
                    "AllGather",
                    mybir.AluOpType.bypass,
                    replica_groups=[list(range(NCORES))],
                    ins=[cc_in.opt()],
                    outs=[cc_gat.opt()],
                )
                nc.scalar.dma_start(out=blk_d, in_=blk[:])

                # MLP weights: loaded on the ACT HWDGE ring, explicitly
                # delayed past the collective input DMA so their bytes ride
                # the collective's dead window instead of the saturated
                # stream or the pre-collective critical path
                w1_sb = sb.tile([RB, CB * HID], F32, name="w1s", tag="w1s", bufs=1)
                w_dmas = [nc.scalar.dma_start(out=w1_sb[:], in_=w1_d)]
                w2_sb = sb.tile([HID, HID], F32, name="w2s", tag="w2s", bufs=1)
                w_dmas.append(nc.scalar.dma_start(out=w2_sb[:], in_=w2_d))
                w3_sb = sb.tile([HID, RB * CB], F32, name="w3s", tag="w3s", bufs=1)
                w_dmas.append(nc.scalar.dma_start(out=w3_sb[:], in_=w3_d))
                b1_sb = sb.tile([HID, 1], F32, name="b1s", tag="b1s", bufs=1)
                w_dmas.append(nc.scalar.dma_start(out=b1_sb[:], in_=b1_d))
                b2_sb = sb.tile([HID, 1], F32, name="b2s", tag="b2s", bufs=1)
                w_dmas.append(nc.scalar.dma_start(out=b2_sb[:], in_=b2_d))
                b3_sb = sb.tile([128, 8], F32, name="b3s", tag="b3s", bufs=1)
                w_dmas.append(nc.scalar.dma_start(out=b3_sb[:], in_=b3_d))
                ic_sb = sb.tile([RB, CB], F32, name="ics", tag="ics", bufs=1)
                w_dmas.append(nc.scalar.dma_start(out=ic_sb[:], in_=ic_d))
                for wd in w_dmas:
                    add_dep_helper(
                        wd.ins, cc_dma.ins, sync=True,
                        reason="weight loads after collective input",
                    )

                # gather all 8 partials [32, (c cb)] with 128B-contiguous runs
                xg = sb.tile([RB, NCORES * CB], F32, name="xg", tag="xg", bufs=1)
                nc.sync.dma_start(
                    out=xg[:].rearrange("p (c q) -> p c q", c=NCORES),
                    in_=cc_gat.rearrange("(c p) q -> p c q", p=RB),
                )
                xs = sb.tile([RB, CB], F32, name="xs", tag="xs", bufs=1)
                nc.vector.tensor_add(xs[:], xg[:, 0:CB], xg[:, CB : 2 * CB])
                for c in range(2, NCORES):
                    nc.vector.tensor_add(
                        xs[:], xs[:], xg[:, CB * c : CB * (c + 1)]
                    )
                nc.vector.tensor_mul(xs[:], xs[:], ic_sb[:])

                # layer 1: h1 = relu(W1.T @ x + b1) as [100, 1] column;
                # contract over rb in 32 cb-chunks: lhsT = W1[rb, cb, :]
                h1p = mps.tile([128, 1], F32, name="h1p", tag="mp", bufs=2)[0:HID, :]
                for cb in range(CB):
                    nc.tensor.matmul(
                        h1p[:], w1_sb[:, HID * cb : HID * (cb + 1)],
                        xs[:, cb : cb + 1],
                        start=(cb == 0), stop=(cb == CB - 1),
                    )
                h1 = sb.tile([HID, 1], F32, name="h1", tag="h1", bufs=1)
                nc.scalar.activation(
                    h1[:], h1p[:], mybir.ActivationFunctionType.Relu, bias=b1_sb[:]
                )
                # layer 2
                h2p = mps.tile([128, 1], F32, name="h2p", tag="mp", bufs=2)[0:HID, :]
                nc.tensor.matmul(h2p[:], w2_sb[:], h1[:], start=True, stop=True)
                h2 = sb.tile([HID, 1], F32, name="h2", tag="h2", bufs=1)
                nc.scalar.activation(
                    h2[:], h2p[:], mybir.ActivationFunctionType.Relu, bias=b2_sb[:]
                )
                # layer 3 + sigmoid, output as [128, 8] columns
                prop = sb.tile([128, 8], F32, name="prop_sb", tag="prop_sb", bufs=1)
                for k in range(8):
                    op = mps.tile([128, 1], F32, name="op", tag="mp", bufs=2)
                    nc.tensor.matmul(
                        op[:], w3_sb[:, 128 * k : 128 * (k + 1)], h2[:],
                        start=True, stop=True,
                    )
                    nc.scalar.activation(
                        prop[:, k : k + 1], op[:],
                        mybir.ActivationFunctionType.Sigmoid,
                        bias=b3_sb[:, k : k + 1],
                    )
                nc.sync.dma_start(out=prop_d, in_=prop[:])
            else:
                nc.sync.dma_start(out=blk_d, in_=blk[:])

    nc.compile()
    return nc


def kernel(X, row_ids, col_ids, W1, b1, W2, b2, W3, b3):
    from concourse.bass_utils import run_bass_kernel_spmd

    X = np.ascontiguousarray(np.asarray(X, dtype=np.float32))
    row_ids = np.asarray(row_ids, dtype=np.int32)
    col_ids = np.asarray(col_ids, dtype=np.int32)
    W1 = np.ascontiguousarray(np.asarray(W1, dtype=np.float32))
    W2 = np.ascontiguousarray(np.asarray(W2, dtype=np.float32))
    W3 = np.ascontiguousarray(np.asarray(W3, dtype=np.float32))
    b1 = np.asarray(b1, dtype=np.float32)
    b2 = np.asarray(b2, dtype=np.float32)
    b3 = np.asarray(b3, dtype=np.float32)

    rcnt = np.bincount(row_ids, minlength=RB).astype(np.int64)
    ccnt = np.bincount(col_ids, minlength=CB).astype(np.int64)
    row_cum = np.concatenate([[0], np.cumsum(rcnt)]).astype(np.int32)
    col_cum = np.concatenate([[0], np.cumsum(ccnt)]).astype(np.int32)

    key = col_cum.tobytes()
    if key not in _cache:
        _cache[key] = _build_program(col_cum)
    nc = _cache[key]

    # host-side index preprocessing
    S = np.zeros((N, RB), dtype=np.float32)
    S[np.arange(N), row_ids] = 1.0
    cnt = np.maximum(
        rcnt[:, None].astype(np.float32) * ccnt[None, :].astype(np.float32), 1.0
    )
    ic_rc = np.ascontiguousarray((1.0 / cnt).astype(np.float32))  # [32, 32]
    b3_col = np.ascontiguousarray(b3.reshape(8, 128).T)          # [128, 8]

    w1r = np.ascontiguousarray(W1.reshape(RB, CB * HID))
    shared = {
        "w1": w1r,
        "w2": W2,
        "w3": W3,
        "b1": np.ascontiguousarray(b1.reshape(HID, 1)),
        "b2": np.ascontiguousarray(b2.reshape(HID, 1)),
        "b3": b3_col,
        "ic": ic_rc,
    }
    in_maps = []
    for c in range(NCORES):
        m = dict(shared)
        m["x"] = np.ascontiguousarray(X[c * RPC : (c + 1) * RPC, :])
        m["s"] = np.ascontiguousarray(S[c * RPC : (c + 1) * RPC, :])
        in_maps.append(m)

    t0 = time.perf_counter()
    try:
        res = run_bass_kernel_spmd(nc, in_maps, core_ids=list(range(NCORES)))
    except ModuleNotFoundError:
        # axon client without the NTFF profiling hook: force trace off
        os.environ["BASS_NEVER_TRACE"] = "1"
        res = run_bass_kernel_spmd(nc, in_maps, core_ids=list(range(NCORES)))
    t1 = time.perf_counter()
    LAST_PERF["exec_time_ns"] = res.exec_time_ns
    LAST_PERF["run_seconds"] = t1 - t0

    if DEVICE_MLP:
        prop_col = res.results[0]["prop"]                         # [128, 8]
        propensity = prop_col.T.reshape(-1).reshape(RB, CB).copy()
    else:
        blk = np.sum([r["blkpart"] for r in res.results], axis=0)
        x_small = (blk / cnt).reshape(-1)
        h = np.maximum(x_small @ W1 + b1, 0.0)
        h = np.maximum(h @ W2 + b2, 0.0)
        o = h @ W3 + b3
        propensity = (1.0 / (1.0 + np.exp(-o))).astype(np.float32).reshape(RB, CB)

    return propensity.astype(np.float32), row_cum, col_cum
